# revision 16
# baseline (speedup 1.0000x reference)
"""Trainium2 Bass kernel for MultiHeadAttention (B=4, S=2048, D=1024, H=16, causal).

Sharding: 8 cores = data-parallel over B (4) x tensor-parallel over heads (2 groups
of 8). Core c handles batch c//2, head group c%2. Per-core dataflow (bf16 matmul
operands with fp32 PSUM accumulation, transposed layouts so no on-chip transposes):

  Qt = (wq_g @ x_q.T + bq_g)      [512, S]   (bias added on eviction, per-partition)
  Kt = (wk_g @ x_k.T + bk_g)      [512, S]
  V  = x_v @ wv_g.T               [S, 8*65]  (ones column per head; bv folded into host bias)
  per head h, query chunk c (512), key tile kt (128), causal:
     E.T[kt] = Kt_h[:,kt].T @ Qt_h[:,c]      [128, 512]   (band tiles causally trimmed)
     P.T = exp(0.125 * E.T)                   (ACT, PSUM->SBUF bf16, causally trimmed)
     P.T *= mask01 on the diagonal block      (GpSimd bf16; DVE stays on evictions)
     O_aug.T += V_aug[kt].T @ P.T            [65, 512]  (row 64 = softmax denom s)
     xh = O.T * bcast(1/s)   (early PSUM evict to SBUF; bcast via K=1 f32r matmul)
     xh -> SBUF-packed x_loc.T tiles [128, S-chunk] (pair of heads per tile)
  fc is split by CONTRACTION rows: partial_out = x_loc.T.T @ wo_loc  [S, 1024]
  (4-matmul chains straight off the local attention outputs -- no collective).
  Host sums the partial outputs of each core pair and adds bo + wo@bv.

The whole pipeline is emitted chunk-major; independent GEMM chains (next chunk's
projections during chunks 0-2, all fc chains during the long final chunk) are
slotted into attention's exp-latency windows so the in-order PE stream never
stalls. Heads run in pairs on disjoint PE row groups (even rows 0-63, odd 64-127)
for concurrent K=64 energies.

Output: per-core bf16 partials [S, 1024]; host adds the pair + bias in f32.
"""

import functools
import sys

import numpy as np

sys.path.insert(0, "/opt/trn_rl_repo")

# --- problem constants (hardcoded; kernel.py must be self-contained) ---
B, S, D, H, HD = 4, 2048, 1024, 16, 64
NCORES = 8
HPC = 8            # heads per core
FLOC = HPC * HD    # 512 local features per core
QCH = 512          # query chunk
KT = 128           # key tile
VW = HD + 1        # V columns per head incl. ones column (65)
NFT = FLOC // 128  # f-tiles per core (4)
NKK = D // 128     # contraction k-tiles (8)
NWARM = 18         # PE warm-up matmuls (cover initial weight/x DMA)
DEBUG_DUMP = False  # add DRAM dumps of Qt/Kt/V tiles (debug only)


def build_program(nc, tile, bass, mybir, seq=S):
    """Emit the per-core SPMD program into `nc` (a Bacc) under a TileContext."""
    dt = mybir.dt
    f32 = dt.float32
    f32r = dt.float32r
    bf16 = dt.bfloat16
    AF = mybir.ActivationFunctionType
    ALU = mybir.AluOpType

    n_tch = seq // QCH          # token chunks
    n_ttile = seq // KT         # 128-token tiles

    # ---- I/O ----
    xqT = nc.dram_tensor("xqT", [D, seq], bf16, kind="ExternalInput").ap()
    xkT = nc.dram_tensor("xkT", [D, seq], bf16, kind="ExternalInput").ap()
    xvT = nc.dram_tensor("xvT", [D, seq], bf16, kind="ExternalInput").ap()
    wqT = nc.dram_tensor("wqT", [D, FLOC], bf16, kind="ExternalInput").ap()
    wkT = nc.dram_tensor("wkT", [D, FLOC], bf16, kind="ExternalInput").ap()
    wvT = nc.dram_tensor("wvT", [D, FLOC], bf16, kind="ExternalInput").ap()
    woT = nc.dram_tensor("woT", [FLOC, D], bf16, kind="ExternalInput").ap()
    bqc = nc.dram_tensor("bqc", [128, NFT], f32, kind="ExternalInput").ap()
    bkc = nc.dram_tensor("bkc", [128, NFT], f32, kind="ExternalInput").ap()
    maskin = nc.dram_tensor("maskin", [KT, KT], bf16, kind="ExternalInput").ap()
    out = nc.dram_tensor("out", [seq, D], bf16, kind="ExternalOutput").ap()

    with tile.TileContext(nc) as tc:
        import contextlib
        ctx = contextlib.ExitStack()
        with ctx:
            # ---------------- pools ----------------
            const = ctx.enter_context(tc.tile_pool(name="const", bufs=1))
            psum = ctx.enter_context(tc.tile_pool(name="psum", bufs=2, space="PSUM"))
            qkv = ctx.enter_context(tc.tile_pool(name="qkv", bufs=1))
            wpool = ctx.enter_context(tc.tile_pool(name="wpool", bufs=1))
            xpool = ctx.enter_context(tc.tile_pool(name="xpool", bufs=32))
            ptpool = ctx.enter_context(tc.tile_pool(name="pt", bufs=4))
            attpool = ctx.enter_context(tc.tile_pool(name="att", bufs=4))
            xhpool = ctx.enter_context(tc.tile_pool(name="xh", bufs=4))
            ostpool = ctx.enter_context(tc.tile_pool(name="ost", bufs=6))

            # ---------------- warm-up tiles (memset, no DMA dep) ----------------
            warm_w = const.tile([128, 128], bf16)
            nc.vector.memset(warm_w[:], 0.0)
            warm_x = const.tile([128, QCH], bf16)
            nc.vector.memset(warm_x[:], 0.0)

            # ---------------- priority DMAs: wq then x_q chunk 0 ----------------
            wts = {}
            for kk in range(NKK):
                wt = wpool.tile([128, FLOC], bf16, tag=f"wq{kk}", name=f"wq{kk}")
                nc.sync.dma_start(wt[:], wqT[kk * 128:(kk + 1) * 128, :])
                wts[("q", kk)] = wt

            def load_x(xsrc, pfx, t):
                tiles = []
                for kk in range(NKK):
                    xt = xpool.tile([128, QCH], bf16, tag="x", name=f"x{pfx}{kk}_{t}")
                    nc.sync.dma_start(
                        xt[:], xsrc[kk * 128:(kk + 1) * 128, t * QCH:(t + 1) * QCH])
                    tiles.append(xt)
                return tiles

            xq = load_x(xqT, "q", 0)

            # PE warm-up: keep TensorE busy while the first DMAs land so the
            # HAM clock-gate opens before real work starts.
            for wi in range(NWARM):
                wp = psum.tile([128, QCH], f32, tag="mm512", name=f"warm{wi}")
                nc.tensor.matmul(wp[:], lhsT=warm_w[:], rhs=warm_x[:],
                                 start=True, stop=True)

            # ---------------- small constants ----------------
            mask_sb = const.tile([KT, KT], bf16)   # 0/1 diagonal-block mask
            nc.sync.dma_start(mask_sb[:], maskin[:])
            bq_sb = const.tile([128, NFT], f32)
            nc.sync.dma_start(bq_sb[:], bqc[:])
            bk_sb = const.tile([128, NFT], f32)
            nc.sync.dma_start(bk_sb[:], bkc[:])
            sel1_f = const.tile([128, HD], f32)
            nc.vector.memset(sel1_f[64:66, :], 1.0)
            sel1 = const.tile([128, HD], f32r)   # rows 64..65 = 1.0 (bcast lhsT)
            nc.vector.tensor_copy(sel1[64:66, :], sel1_f[64:66, :])

            # ---------------- remaining weights + chunk-0 x ----------------
            for wname, wsrc in (("k", wkT), ("v", wvT)):
                for kk in range(NKK):
                    wt = wpool.tile([128, FLOC], bf16, tag=f"w{wname}{kk}",
                                    name=f"w{wname}{kk}")
                    nc.sync.dma_start(wt[:], wsrc[kk * 128:(kk + 1) * 128, :])
                    wts[(wname, kk)] = wt
                if wname == "k":
                    xk = load_x(xkT, "k", 0)
            xv = load_x(xvT, "v", 0)
            wo_sb = []
            for hp in range(NFT):
                wt = wpool.tile([128, D], bf16, tag=f"wo{hp}", name=f"wo{hp}")
                nc.sync.dma_start(wt[:], woT[hp * 128:(hp + 1) * 128, :])
                wo_sb.append(wt)

            # persistent projection outputs
            qt_tiles = [qkv.tile([128, seq], bf16, tag=f"qt{i}", name=f"qt{i}")
                        for i in range(NFT)]
            kt_tiles = [qkv.tile([128, seq], bf16, tag=f"kt{i}", name=f"kt{i}")
                        for i in range(NFT)]
            v_tiles = [qkv.tile([KT, HPC * VW], bf16, tag=f"v{i}", name=f"v{i}")
                       for i in range(n_ttile)]
            xh_tiles = {}   # (chunk, hp) -> packed [128, QCH] bf16 x_loc.T tile

            def proj_qk_parts(pfx, xts, bias_sb, dst, t, f):
                """Two fillers (4+4 matmuls) for one Q/K projection chain."""
                box = {}

                def a():
                    pp = psum.tile([128, QCH], f32, tag="mm512",
                                   name=f"pp{pfx}{t}{f}")
                    box["pp"] = pp
                    for kk in range(NKK // 2):
                        nc.tensor.matmul(
                            pp[:], lhsT=wts[(pfx, kk)][:, f * 128:(f + 1) * 128],
                            rhs=xts[kk][:], start=(kk == 0), stop=False,
                            skip_group_check=True)

                def b():
                    pp = box["pp"]
                    for kk in range(NKK // 2, NKK):
                        nc.tensor.matmul(
                            pp[:], lhsT=wts[(pfx, kk)][:, f * 128:(f + 1) * 128],
                            rhs=xts[kk][:], start=False, stop=(kk == NKK - 1),
                            skip_group_check=True)
                    nc.vector.tensor_scalar_add(
                        dst[f][:, t * QCH:(t + 1) * QCH], pp[:],
                        bias_sb[:, f:f + 1])

                return [a, b]

            def proj_v_parts(xts, t, tt):
                g = t * (QCH // KT) + tt
                box = {}

                def a():
                    pp = psum.tile([128, FLOC], f32, tag="mm512", name=f"ppv{g}")
                    box["pp"] = pp
                    for kk in range(NKK // 2):
                        nc.tensor.matmul(
                            pp[:], lhsT=xts[kk][:, tt * KT:(tt + 1) * KT],
                            rhs=wts[("v", kk)][:], start=(kk == 0), stop=False,
                            skip_group_check=True)

                def b():
                    pp = box["pp"]
                    for kk in range(NKK // 2, NKK):
                        nc.tensor.matmul(
                            pp[:], lhsT=xts[kk][:, tt * KT:(tt + 1) * KT],
                            rhs=wts[("v", kk)][:], start=False,
                            stop=(kk == NKK - 1), skip_group_check=True)
                    vv = v_tiles[g].rearrange("p (h e) -> p h e", e=VW)
                    nc.vector.tensor_copy(
                        vv[:, :, 0:HD], pp[:].rearrange("p (h d) -> p h d", d=HD))
                    # ones column via DVE memset: same engine as the V copy, so
                    # the two writers serialize (a DMA here would RMW-tear the
                    # shared 16B SBUF lines while the copy is in flight)
                    nc.vector.memset(vv[:, :, HD:VW], 1.0)

                return [a, b]

            def proj_chunk_fillers(t, xq, xk, xv):
                fillers = []
                for f in range(NFT):
                    fillers.extend(proj_qk_parts("q", xq, bq_sb, qt_tiles, t, f))
                for f in range(NFT):
                    fillers.extend(proj_qk_parts("k", xk, bk_sb, kt_tiles, t, f))
                for tt in range(QCH // KT):
                    fillers.extend(proj_v_parts(xv, t, tt))
                return fillers

            def attention_pair(c, hp, fill=None):
                ft = hp
                heads = (2 * hp, 2 * hp + 1)
                pvs, eps, pts = {}, {}, {}
                xh_t = xhpool.tile([128, QCH], bf16, tag=f"xh{hp}",
                                   name=f"xh{c}_{hp}")
                xh_tiles[(c, hp)] = xh_t
                for h in heads:
                    pvs[h] = psum.tile([VW, QCH], f32, tag=f"pv{h % 2}",
                                       name=f"pv{c}_{h}", bufs=1)
                nkt = (QCH // KT) * (c + 1)     # causal key tiles
                for grp in range(nkt // 2):
                    for h in heads:
                        eps[h] = psum.tile([128, 2 * QCH], f32,
                                           tag=f"epair{h % 2}",
                                           name=f"ep{c}_{h}_{grp}", bufs=1)
                    # kt-interleaved: adjacent matmuls hit disjoint PE row-groups
                    offs = []
                    for j2 in range(2):
                        kt = grp * 2 + j2
                        band = kt - (QCH // KT) * c
                        offs.append(band * KT if band > 0 else 0)   # causal trim
                        for h in heads:
                            fr = (h % 2) * HD
                            nc.tensor.matmul(
                                eps[h][:, j2 * QCH + offs[j2]:(j2 + 1) * QCH],
                                lhsT=kt_tiles[ft][fr:fr + HD,
                                                  kt * KT:(kt + 1) * KT],
                                rhs=qt_tiles[ft][fr:fr + HD,
                                                 c * QCH + offs[j2]:(c + 1) * QCH],
                                start=True, stop=True)
                    for h in heads:
                        pt = ptpool.tile([128, 2 * QCH], bf16, tag=f"pt{h % 2}",
                                         name=f"pt{c}_{h}_{grp}")
                        pts[h] = pt
                        if offs[1] > 0:   # causally trimmed: ACT per key tile
                            nc.scalar.activation(
                                pt[:, offs[0]:QCH], eps[h][:, offs[0]:QCH],
                                AF.Exp, scale=0.125)
                            nc.scalar.activation(
                                pt[:, QCH + offs[1]:2 * QCH],
                                eps[h][:, QCH + offs[1]:2 * QCH],
                                AF.Exp, scale=0.125)
                        else:
                            nc.scalar.activation(pt[:], eps[h][:], AF.Exp,
                                                 scale=0.125)
                        for j2 in range(2):
                            kt = grp * 2 + j2
                            band = kt - (QCH // KT) * c
                            if band >= 0:   # mask the diagonal block
                                sl = pt[:, j2 * QCH + band * KT:
                                        j2 * QCH + (band + 1) * KT]
                                nc.vector.tensor_tensor(sl, sl, mask_sb[:],
                                                        ALU.mult)
                    for j2 in range(2):
                        kt = grp * 2 + j2
                        off = offs[j2]
                        for h in heads:
                            nc.tensor.matmul(
                                pvs[h][:, off:QCH],
                                lhsT=v_tiles[kt][:, :].rearrange(
                                    "p (h e) -> p h e", e=VW)[:, h, :],
                                rhs=pts[h][:, j2 * QCH + off:(j2 + 1) * QCH],
                                start=(kt == 0), stop=(kt == nkt - 1),
                                skip_group_check=True)
                    if fill is not None:
                        fill()   # slot one indep. GEMM chain into the exp window
                # normalize: evict PSUM early, then xh = O.T * bcast(1/s)
                for h in heads:
                    pv = pvs[h]
                    ao = attpool.tile([VW, QCH], f32, tag=f"ao{h % 2}",
                                      name=f"ao{c}_{h}")
                    nc.vector.tensor_copy(ao[:], pv[:])   # frees the pv bank
                    sr = attpool.tile([128, QCH], f32r, tag="sr", name=f"sr{c}_{h}")
                    nc.vector.tensor_copy(sr[64:65, :], ao[HD:VW, :])
                    bc = psum.tile([HD, QCH], f32, tag=f"pv{h % 2}",
                                   name=f"bc{c}_{h}", bufs=1)
                    nc.tensor.matmul(bc[:], lhsT=sel1[64:65, :], rhs=sr[64:65, :],
                                     start=True, stop=True)
                    rcp = attpool.tile([HD, QCH], f32, tag="rcp", name=f"rcp{c}_{h}")
                    nc.vector.reciprocal_approx_fast(rcp[:], bc[:])
                    xh = attpool.tile([HD, QCH], bf16, tag="xhs", name=f"xhs{c}_{h}")
                    nc.vector.tensor_tensor(xh[:], ao[0:HD, :], rcp[:], ALU.mult)
                    nc.sync.dma_start(
                        xh_t[(h % 2) * HD:(h % 2 + 1) * HD, :], xh[:])

            def fc_chain_parts(c, tt, half, tag="mm512"):
                """fc chain for (chunk, token tile, feature half) as 2 fillers."""
                box = {}

                def a():
                    kw = {} if tag == "mm512" else {"bufs": 1}
                    fp = psum.tile([128, QCH], f32, tag=tag,
                                   name=f"fp{c}_{tt}_{half}", **kw)
                    box["fp"] = fp
                    for hp in range(NFT // 2):
                        nc.tensor.matmul(
                            fp[:], lhsT=xh_tiles[(c, hp)][:, tt * KT:(tt + 1) * KT],
                            rhs=wo_sb[hp][:, half * QCH:(half + 1) * QCH],
                            start=(hp == 0), stop=False, skip_group_check=True)

                def b():
                    fp = box["fp"]
                    for hp in range(NFT // 2, NFT):
                        nc.tensor.matmul(
                            fp[:], lhsT=xh_tiles[(c, hp)][:, tt * KT:(tt + 1) * KT],
                            rhs=wo_sb[hp][:, half * QCH:(half + 1) * QCH],
                            start=False, stop=(hp == NFT - 1),
                            skip_group_check=True)
                    ost = ostpool.tile([128, QCH], bf16, tag="ost",
                                       name=f"ost{c}_{tt}_{half}")
                    nc.vector.tensor_copy(ost[:], fp[:])
                    # store in 4 slices so the DMAs spread across queues
                    # (a single [128,512] store is 128 descriptors on one queue
                    #  ~4-8us; quarters overlap and shrink the exposed tail)
                    for q4 in range(4):
                        nc.sync.dma_start(
                            out[c * QCH + tt * KT + q4 * 32:
                                c * QCH + tt * KT + (q4 + 1) * 32,
                                half * QCH:(half + 1) * QCH],
                            ost[q4 * 32:(q4 + 1) * 32, :])

                return [a, b]

            def fc_chunk_fillers(c):
                fillers = []
                for tt in range(QCH // KT):
                    for half in range(2):
                        fillers.extend(fc_chain_parts(c, tt, half))
                return fillers

            # ---------------- software-pipelined chunk loop ----------------
            # proj(t+1) chains (chunks 0-2) and ALL fc chains (chunk 3, which has
            # the most attention groups) are slotted into attention's exp-latency
            # windows so the in-order PE stream never stalls.
            import collections as _cl
            for fl in proj_chunk_fillers(0, xq, xk, xv):
                fl()   # prologue: chunk 0 projections up front
            for t in range(n_tch):
                fillers = _cl.deque()
                if t + 1 < n_tch:
                    xq = load_x(xqT, "q", t + 1)
                    xk = load_x(xkT, "k", t + 1)
                    xv = load_x(xvT, "v", t + 1)
                    fillers.extend(proj_chunk_fillers(t + 1, xq, xk, xv))
                else:
                    for tc_prev in range(n_tch - 1):
                        fillers.extend(fc_chunk_fillers(tc_prev))

                nfill = 2 if t == 0 else 1   # chunk 0 has spare fillers

                def fill(fillers=fillers, nfill=nfill):
                    for _ in range(nfill):
                        if fillers:
                            fillers.popleft()()

                for hp in range(HPC // 2):
                    attention_pair(t, hp, fill=fill)
                while fillers:
                    fillers.popleft()()
            # epilogue: last chunk's fc. Spread the first 6 chains over the
            # now-idle attention PSUM banks and emit all their first halves
            # up front, so the PE streams while the last pair normalizes.
            ep_tags = ["mm512", "mm512", "epair0", "epair1", "pv0", "pv1"]
            lc = n_tch - 1
            combos = [(tt, half) for tt in range(QCH // KT) for half in range(2)]
            spread = [fc_chain_parts(lc, tt, half, tag=ep_tags[i])
                      for i, (tt, half) in enumerate(combos[:6])]
            for pa, _ in spread:
                pa()
            for _, pb in spread:
                pb()
            for tt, half in combos[6:]:
                pa, pb = fc_chain_parts(lc, tt, half)
                pa()
                pb()

            if DEBUG_DUMP:
                dbg_q = nc.dram_tensor("dbg_q", [NFT * 128, seq], bf16,
                                       kind="ExternalOutput").ap()
                dbg_k = nc.dram_tensor("dbg_k", [NFT * 128, seq], bf16,
                                       kind="ExternalOutput").ap()
                dbg_v = nc.dram_tensor("dbg_v", [n_ttile * KT, HPC * VW], bf16,
                                       kind="ExternalOutput").ap()
                dbg_xh = nc.dram_tensor("dbg_xh", [n_tch * NFT * 128, QCH],
                                        bf16, kind="ExternalOutput").ap()
                for i in range(NFT):
                    nc.sync.dma_start(dbg_q[i * 128:(i + 1) * 128, :],
                                      qt_tiles[i][:])
                    nc.sync.dma_start(dbg_k[i * 128:(i + 1) * 128, :],
                                      kt_tiles[i][:])
                for g in range(n_ttile):
                    nc.sync.dma_start(dbg_v[g * KT:(g + 1) * KT, :],
                                      v_tiles[g][:])
                for (c, hp), xt in xh_tiles.items():
                    r = (c * NFT + hp) * 128
                    nc.sync.dma_start(dbg_xh[r:r + 128, :], xt[:])
    return nc


@functools.lru_cache(maxsize=None)
def _compiled(seq=S):
    import concourse.bacc as bacc
    import concourse.bass as bass
    import concourse.mybir as mybir
    import concourse.tile as tile

    nc = bacc.Bacc("TRN2", target_bir_lowering=False, debug=False,
                   num_devices=NCORES)
    build_program(nc, tile, bass, mybir, seq=seq)
    nc.compile()
    return nc


def _host_prep(inputs, seq=S):
    """Build the 8 per-core input maps from full inputs."""
    import ml_dtypes
    bf16 = ml_dtypes.bfloat16

    q, k, v = inputs["query"], inputs["key"], inputs["value"]
    wq, bq = inputs["wq"], inputs["bq"]
    wk, bk = inputs["wk"], inputs["bk"]
    wv = inputs["wv"]
    wo = inputs["wo"]

    f32 = np.float32

    # 0/1 diagonal-block mask [128, 128]
    kk = np.arange(KT)[:, None]
    qq = np.arange(KT)[None, :]
    mask = (qq >= kk).astype(bf16)

    in_maps = []
    for core in range(NCORES):
        b, g = core // 2, core % 2
        sl = slice(g * FLOC, (g + 1) * FLOC)
        in_maps.append({
            "xqT": np.ascontiguousarray(q[b, :seq].T).astype(bf16),
            "xkT": np.ascontiguousarray(k[b, :seq].T).astype(bf16),
            "xvT": np.ascontiguousarray(v[b, :seq].T).astype(bf16),
            "wqT": np.ascontiguousarray(wq[sl].T).astype(bf16),
            "wkT": np.ascontiguousarray(wk[sl].T).astype(bf16),
            "wvT": np.ascontiguousarray(wv[sl].T).astype(bf16),
            "woT": np.ascontiguousarray(wo[:, sl].T).astype(bf16),
            "bqc": np.ascontiguousarray(bq[sl].reshape(NFT, 128).T).astype(f32),
            "bkc": np.ascontiguousarray(bk[sl].reshape(NFT, 128).T).astype(f32),
            "maskin": mask,
        })
    return in_maps


def run(inputs, seq=S, trace=False):
    from concourse.bass_utils import run_bass_kernel_spmd

    nc = _compiled(seq)
    in_maps = _host_prep(inputs, seq)
    res = run_bass_kernel_spmd(nc, in_maps, core_ids=list(range(NCORES)),
                               trace=trace)
    bo_eff = (inputs["bo"] + inputs["wo"] @ inputs["bv"]).astype(np.float32)
    out = np.zeros((B, seq, D), np.float32)
    for b in range(B):
        out[b] = (res.results[2 * b]["out"].astype(np.float32)
                  + res.results[2 * b + 1]["out"].astype(np.float32)
                  + bo_eff[None, :])
    return out, res


def kernel(**inputs):
    inputs = {k: np.asarray(v) for k, v in inputs.items()}
    out, _ = run(inputs)
    return out
